# revision 2
# baseline (speedup 1.0000x reference)
"""AttentionalSplatting Trainium2 kernel (8 NeuronCores, SPMD).

Sharding: 8 cores = T(4) x HW-halves(2).  Core c handles t = c//2 and pixel
columns [ (c%2)*1152, (c%2+1)*1152 ).  Each core runs the full pipeline for
its (t, pixel-half): coord-proj + 2D RoPE -> Q/K/V proj -> qk-norm ->
scores(+spatial bias) -> softmax -> attend -> Wo -> W_out -> residual.
No cross-core communication is needed; outputs concatenate.

On-chip layout is feature-major ("transposed"): feature/head dims live on
SBUF partitions, pixels/tokens on the free dim.  Scores are computed as
S^T (m on partitions, q free) so the attend matmul consumes exp(S^T)
directly and softmax sums arrive free via a ones-column appended to V.
The spatial bias enters PSUM through identity matmuls so the exp can read
(scores+bias) straight out of PSUM on the scalar engine.
"""

import math
import sys

import numpy as np

sys.path.insert(0, "/opt/trn_rl_repo")

import ml_dtypes  # noqa: E402

import concourse.bass as bass  # noqa: E402
import concourse.bacc as bacc  # noqa: E402
import concourse.tile as tile  # noqa: E402
from concourse import mybir  # noqa: E402
from concourse.bass_utils import run_bass_kernel_spmd  # noqa: E402

T, M, HW, DF, H = 4, 1024, 2304, 256, 8
DKH = DF // H  # 32
QH = HW // 2  # 1152 pixels per core
SCALE = 1.0 / math.sqrt(DKH)
D_HALF = DF // 2  # 128
D_QUART = DF // 4  # 64
THETA = (100.0 ** (-2.0 * np.arange(D_QUART, dtype=np.float32) / D_HALF)).astype(
    np.float32
)

F32 = mybir.dt.float32
BF16 = mybir.dt.bfloat16
AF = mybir.ActivationFunctionType
BF16NP = ml_dtypes.bfloat16

N_CORES = 8
Q_BLOCKS = [(0, 512), (512, 512), (1024, 128)]
M_TRIPS = [(0, 1, 2), (3, 4, 5), (6, 7)]
K_CHUNKS = [(0, 512), (512, 512)]


def _bf(x):
    return np.ascontiguousarray(np.asarray(x, np.float32)).astype(BF16NP)


def _f32(x):
    return np.ascontiguousarray(np.asarray(x, np.float32))


def _host_constants(Wq, Wk, Wv, Wo, W_out_w, W_out_b, W_coord_w, W_coord_b):
    """Shared (core-independent) device constants, host-precomputed."""
    # pair-swapped coord weights for RoPE (swap even/odd output columns)
    perm = np.arange(DF)
    perm = perm.reshape(-1, 2)[:, ::-1].reshape(-1)
    wcsw = W_coord_w[:, perm]
    wcbsw = W_coord_b[perm]
    # signed duplicated theta: [-t0, +t0, -t1, +t1, ...]
    th = np.empty((1, D_HALF), np.float32)
    th[0, 0::2] = -THETA
    th[0, 1::2] = THETA
    # block-ones for per-head sum of squares: dtile k maps its 128 feature
    # rows onto head columns 4k..4k+3
    bones = np.zeros((2, 128, 8), np.float32)
    for k in range(2):
        for d in range(128):
            bones[k, d, 4 * k + d // 32] = 1.0
    # expand per-head scalars (8, q) back to the 128 feature rows of dtile k
    exp8 = np.zeros((2, 8, 128), np.float32)
    for k in range(2):
        for d in range(128):
            exp8[k, 4 * k + d // 32, d] = 1.0
    # expand per-head inv-sum (8, q) to paired attend-output rows:
    # pair j holds head 2j at rows 1..33 and head 2j+1 at rows 65..97
    expP = np.zeros((4, 8, 128), np.float32)
    for j in range(4):
        expP[j, 2 * j, 1:33] = 1.0
        expP[j, 2 * j + 1, 65:97] = 1.0
    # Wo rearranged to the paired attend-output row layout (sumexp rows = 0)
    wo_aug = np.zeros((4, 128, DF), np.float32)
    for j in range(4):
        wo_aug[j, 1:33, :] = Wo[(2 * j) * 32 : (2 * j + 1) * 32, :]
        wo_aug[j, 65:97, :] = Wo[(2 * j + 1) * 32 : (2 * j + 2) * 32, :]
    return {
        "wq": _bf(Wq),
        "wk": _bf(Wk),
        "wv": _bf(Wv),
        "wo_aug": _bf(wo_aug),
        "wout": _bf(W_out_w),
        "woutb": _f32(W_out_b).reshape(DF, 1),
        "wc": _f32(W_coord_w),
        "wcb": _f32(W_coord_b).reshape(DF, 1),
        "wcsw": _f32(wcsw),
        "wcbsw": _f32(wcbsw).reshape(DF, 1),
        "theta2s": th,
        "bones": bones,
        "exp8": exp8,
        "expP": expP,
        "ident": _bf(np.eye(128, dtype=np.float32)),
    }


_NC_CACHE = None


def _build_nc():
    nc = bacc.Bacc(
        "TRN2",
        target_bir_lowering=False,
        debug=False,
        enable_asserts=True,
        num_devices=N_CORES,
    )
    d = {}
    inp = lambda name, shape, dt: d.__setitem__(
        name, nc.declare_dram_parameter(name, list(shape), dt, isOutput=False)
    )
    inp("tokT", (DF, M), BF16)
    inp("posT", (2, QH), F32)
    inp("biasT", (M, QH), BF16)
    inp("fmapT", (DF, QH), F32)
    inp("wq", (DF, DF), BF16)
    inp("wk", (DF, DF), BF16)
    inp("wv", (DF, DF), BF16)
    inp("wo_aug", (4, 128, DF), BF16)
    inp("wout", (DF, DF), BF16)
    inp("woutb", (DF, 1), F32)
    inp("wc", (2, DF), F32)
    inp("wcb", (DF, 1), F32)
    inp("wcsw", (2, DF), F32)
    inp("wcbsw", (DF, 1), F32)
    inp("theta2s", (1, D_HALF), F32)
    inp("bones", (2, 128, 8), F32)
    inp("exp8", (2, 8, 128), F32)
    inp("expP", (4, 8, 128), F32)
    inp("ident", (128, 128), BF16)
    out = nc.declare_dram_parameter("out", [DF, QH], F32, isOutput=True)

    import os as _os

    with tile.TileContext(
        nc, trace_sim=bool(_os.environ.get("KERNEL_TRACE_SIM"))
    ) as tc:
        _body(nc, tc, d, out)
    nc.compile()
    return nc


def _body(nc, tc, d, out_dram):
    mm = nc.tensor.matmul
    act = nc.scalar.activation
    dma = nc.sync.dma_start

    with (
        tc.tile_pool(name="const", bufs=1) as cpool,
        tc.tile_pool(name="work", bufs=1) as wpool,
        tc.tile_pool(name="persist", bufs=1) as ppool,
        tc.tile_pool(name="epool", bufs=4) as epool,
        tc.tile_pool(name="psA", bufs=2, space=bass.MemorySpace.PSUM) as psA,
        tc.tile_pool(name="psB", bufs=2, space=bass.MemorySpace.PSUM) as psB,
    ):
        # ---- load constants / inputs to SBUF ----
        # 256-row tensors fold to (128, 2, ...): [:, kt, ...] = rows kt*128..
        def load(name, shape, dt, rearrange=None, **kw):
            t = cpool.tile(list(shape), dt, tag=name)
            src = d[name][:]
            if rearrange is not None:
                src = src.rearrange(rearrange, **kw)
            dma(t[:], src)
            return t

        fold = "(k p) d -> p k d"
        wq = load("wq", (128, 2, DF), BF16, fold, p=128)
        wk = load("wk", (128, 2, DF), BF16, fold, p=128)
        wv = load("wv", (128, 2, DF), BF16, fold, p=128)
        wo_aug = load("wo_aug", (128, 4, DF), BF16, "j p d -> p j d")
        wout = load("wout", (128, 2, DF), BF16, fold, p=128)
        woutb = load("woutb", (128, 2, 1), F32, fold, p=128)
        wc = load("wc", (2, DF), F32)
        wcb = load("wcb", (128, 2, 1), F32, fold, p=128)
        wcsw = load("wcsw", (2, DF), F32)
        wcbsw = load("wcbsw", (128, 2, 1), F32, fold, p=128)
        th2 = load("theta2s", (1, D_HALF), F32)
        bones = load("bones", (128, 2, 8), F32, "k p h -> p k h")
        exp8 = load("exp8", (8, 2, 128), F32, "k h d -> h k d")
        expP = load("expP", (8, 4, 128), F32, "j s e -> s j e")
        ident = load("ident", (128, 128), BF16)
        posT = load("posT", (2, QH), F32)
        posT2 = cpool.tile([1, 2, QH], F32, tag="posT2")
        dma(posT2[:], d["posT"][:].rearrange("(o a) q -> o a q", o=1))
        fmapT = load("fmapT", (128, 2, QH), F32, fold, p=128)
        tokT = load("tokT", (128, 2, M), BF16, fold, p=128)

        bias_sb = []
        for mc in range(8):
            bt = ppool.tile([128, QH], BF16, tag=f"bias{mc}")
            dma(bt[:], d["biasT"][mc * 128 : (mc + 1) * 128, :])
            bias_sb.append(bt)

        def const_tile(val, name):
            t = cpool.tile([128, 1], F32, tag=name)
            nc.vector.memset(t[:], val)
            return t

        halfpi = const_tile(math.pi / 2.0, "halfpi")
        zero_c = const_tile(0.0, "zeroc")
        lnscale = const_tile(math.log(SCALE), "lnscale")

        # ---- Q path: Qin^T (and pair-swapped) = Wc^T @ pos^T (+b) ----
        qin = []
        for dt_i, (w, b) in enumerate(((wc, wcb), (wcsw, wcbsw))):
            for half in range(2):
                ps = psA.tile([128, QH], F32, tag="big")
                for qo, qb in Q_BLOCKS:
                    mm(
                        ps[:, qo : qo + qb],
                        w[:, half * 128 : (half + 1) * 128],
                        posT[:, qo : qo + qb],
                    )
                t = wpool.tile([128, QH], BF16, tag=f"qin{dt_i}{half}")
                nc.vector.tensor_scalar_add(t[:], ps[:], b[:, half, :])
                qin.append(t)
        qin0, qin1, qins0, qins1 = qin

        # ---- RoPE tables: one angle matmul per axis (signed theta) ----
        cs = []
        for axis in range(2):
            ps = psA.tile([128, QH], F32, tag="big")
            for qo, qb in Q_BLOCKS:
                mm(
                    ps[:, qo : qo + qb],
                    th2[:, :],
                    posT2[:, axis, qo : qo + qb],
                )
            c_t = wpool.tile([128, QH], BF16, tag=f"cos{axis}")
            s_t = wpool.tile([128, QH], BF16, tag=f"sin{axis}")
            act(c_t[:], ps[:], AF.Sin, bias=halfpi[:])
            act(s_t[:], ps[:], AF.Sin)
            cs.append((c_t, s_t))

        roped = []
        for dt_i, (q, qs) in enumerate(((qin0, qins0), (qin1, qins1))):
            c_t, s_t = cs[dt_i]
            t1 = wpool.tile([128, QH], BF16, tag=f"ropea{dt_i}")
            nc.vector.tensor_mul(t1[:], q[:], c_t[:])
            t2 = wpool.tile([128, QH], BF16, tag=f"ropeb{dt_i}")
            nc.vector.tensor_mul(t2[:], qs[:], s_t[:])
            r = wpool.tile([128, QH], BF16, tag=f"roped{dt_i}")
            nc.vector.tensor_add(r[:], t1[:], t2[:])
            roped.append(r)

        # ---- Q = roped @ Wq  (computed as Q^T, feature-major) ----
        def proj_T(w_sb, rhs_tiles, n, blocks, name):
            """out^T[dt] (128, n) = sum_kt w[kt,dt]^T @ rhs[kt]; returns psum tiles"""
            outs = []
            for dt_i in range(2):
                ps = psA.tile([128, n], F32, tag="big")
                for qo, qb in blocks:
                    for kt in range(2):
                        mm(
                            ps[:, qo : qo + qb],
                            w_sb[:, kt, dt_i * 128 : (dt_i + 1) * 128],
                            rhs_tiles[kt][:, qo : qo + qb],
                            start=(kt == 0),
                            stop=(kt == 1),
                        )
                outs.append(ps)
            return outs

        def qknorm(ps_list, n, blocks, ln_bias, name):
            """psum (128, n) x2 -> normalized bf16 tiles (128, n) x2"""
            sq_ps = psA.tile([8, n], F32, tag="big")
            bf_tiles = []
            for dt_i, ps in enumerate(ps_list):
                tb = ppool.tile([128, n], BF16, tag=f"{name}n{dt_i}")
                nc.vector.tensor_copy(tb[:], ps[:])
                sq = wpool.tile([128, n], F32, tag=f"{name}f{dt_i}")
                nc.vector.tensor_copy(sq[:], ps[:])
                nc.vector.tensor_mul(sq[:], sq[:], sq[:])
                for qo, qb in blocks:
                    mm(
                        sq_ps[:, qo : qo + qb],
                        bones[:, dt_i, :],
                        sq[:, qo : qo + qb],
                        start=(dt_i == 0),
                        stop=(dt_i == 1),
                    )
                bf_tiles.append(tb)
            lnt = wpool.tile([8, n], F32, tag=f"{name}ln")
            act(lnt[:], sq_ps[:], AF.Ln)
            if ln_bias is None:
                ln_bias = zero_c
            invn = wpool.tile([8, n], F32, tag=f"{name}inv")
            act(invn[:], lnt[:], AF.Exp, scale=-0.5, bias=ln_bias[:8, :])
            outs = []
            for dt_i, tb in enumerate(bf_tiles):
                psx = psA.tile([128, n], F32, tag="big")
                for qo, qb in blocks:
                    mm(psx[:, qo : qo + qb], exp8[:, dt_i, :], invn[:, qo : qo + qb])
                tn = ppool.tile([128, n], BF16, tag=f"{name}T{dt_i}")
                nc.vector.tensor_mul(tn[:], tb[:], psx[:])
                outs.append(tn)
            return outs

        q_ps = proj_T(wq, roped, QH, Q_BLOCKS, "q")
        qnT = qknorm(q_ps, QH, Q_BLOCKS, lnscale, "q")

        tok_tiles = [tokT[:, 0, :], tokT[:, 1, :]]
        k_ps = proj_T(wk, tok_tiles, M, K_CHUNKS, "k")
        knT = qknorm(k_ps, M, K_CHUNKS, None, "k")

        # ---- V (token-major) with ones column:  vsb[mc] = (128, 8, 33) ----
        vsb = []
        for mc in range(8):
            ps = psB.tile([128, 256], F32, tag="small")
            for kt in range(2):
                mm(
                    ps[:],
                    tokT[:, kt, mc * 128 : (mc + 1) * 128],
                    wv[:, kt, :],
                    start=(kt == 0),
                    stop=(kt == 1),
                )
            vt = ppool.tile([128, 8, 33], BF16, tag=f"v{mc}")
            nc.vector.memset(vt[:, :, 0:1], 1.0)
            nc.vector.tensor_copy(
                vt[:, :, 1:33], ps[:].rearrange("p (h e) -> p h e", h=8)
            )
            vsb.append(vt)

        # ---- main attention loop ----
        # pair j: head 2j accumulates at psum rows 0..32, head 2j+1 at 64..96
        osb = []  # per pair (128, QH) bf16, rows 0/64 = sumexp
        for j in range(4):
            t = ppool.tile([128, QH], BF16, tag=f"osb{j}")
            osb.append(t)

        for qo, qb in Q_BLOCKS:
            for j in range(4):
                heads = (2 * j, 2 * j + 1)
                o_ps = psB.tile([128, qb], F32, tag="small")
                for trip in M_TRIPS:
                    w3 = len(trip) * qb
                    e_ts = {}
                    s_tiles = {}
                    for h in heads:
                        dt_i = h // 4
                        hp = (h % 4) * 32
                        s_ps = psA.tile([128, w3], F32, tag="big")
                        s_tiles[h] = s_ps
                        for i, mc in enumerate(trip):
                            mm(
                                s_ps[:, i * qb : (i + 1) * qb],
                                ident[:],
                                bias_sb[mc][:, qo : qo + qb],
                                start=True,
                                stop=False,
                            )
                    for i, mc in enumerate(trip):
                        for h in heads:
                            dt_i = h // 4
                            hp = (h % 4) * 32
                            mm(
                                s_tiles[h][:, i * qb : (i + 1) * qb],
                                knT[dt_i][hp : hp + 32, mc * 128 : (mc + 1) * 128],
                                qnT[dt_i][hp : hp + 32, qo : qo + qb],
                                start=False,
                                stop=True,
                                tile_position=(hp, 0),
                            )
                    for h in heads:
                        e_t = epool.tile([128, 3 * qb], BF16, tag="E")
                        act(e_t[:, 0:w3], s_tiles[h][:], AF.Exp)
                        e_ts[h] = e_t
                    for i, mc in enumerate(trip):
                        for h in heads:
                            base = 64 * (h % 2)
                            mm(
                                o_ps[base : base + 33, :],
                                vsb[mc][:, h, :],
                                e_ts[h][:, i * qb : (i + 1) * qb],
                                start=(mc == 0),
                                stop=(mc == 7),
                                tile_position=(0, base),
                            )
                nc.vector.tensor_copy(osb[j][:, qo : qo + qb], o_ps[:])

        # ---- softmax denominators: gather row 0 of each head, invert ----
        sumE = wpool.tile([8, QH], BF16, tag="sumE")
        for h in range(8):
            j, r = h // 2, 64 * (h % 2)
            dma(sumE[h : h + 1, :], osb[h // 2][r : r + 1, :])
        lnS = wpool.tile([8, QH], F32, tag="lnS")
        act(lnS[:], sumE[:], AF.Ln)
        invS = wpool.tile([8, QH], F32, tag="invS")
        act(invS[:], lnS[:], AF.Exp, scale=-1.0)

        for j in range(4):
            for qo, qb in Q_BLOCKS:
                ps = psB.tile([128, qb], F32, tag="small")
                mm(ps[:], expP[:, j, :], invS[:, qo : qo + qb])
                nc.vector.tensor_mul(
                    osb[j][:, qo : qo + qb], osb[j][:, qo : qo + qb], ps[:]
                )

        # ---- output projections + residual ----
        o1b = []
        for dt_i in range(2):
            ps = psA.tile([128, QH], F32, tag="big")
            for qo, qb in Q_BLOCKS:
                for j in range(4):
                    mm(
                        ps[:, qo : qo + qb],
                        wo_aug[:, j, dt_i * 128 : (dt_i + 1) * 128],
                        osb[j][:, qo : qo + qb],
                        start=(j == 0),
                        stop=(j == 3),
                    )
            t = wpool.tile([128, QH], BF16, tag=f"o1b{dt_i}")
            nc.vector.tensor_copy(t[:], ps[:])
            o1b.append(t)

        for dt_i in range(2):
            ps = psA.tile([128, QH], F32, tag="big")
            for qo, qb in Q_BLOCKS:
                for kt in range(2):
                    mm(
                        ps[:, qo : qo + qb],
                        wout[:, kt, dt_i * 128 : (dt_i + 1) * 128],
                        o1b[kt][:, qo : qo + qb],
                        start=(kt == 0),
                        stop=(kt == 1),
                    )
            r1 = wpool.tile([128, QH], F32, tag=f"res{dt_i}")
            nc.vector.tensor_scalar_add(r1[:], ps[:], woutb[:, dt_i, :])
            nc.vector.tensor_add(r1[:], r1[:], fmapT[:, dt_i, :])
            dma(out_dram[dt_i * 128 : (dt_i + 1) * 128, :], r1[:])


def _prep_in_maps(inputs):
    consts = _host_constants(
        np.asarray(inputs["Wq"], np.float32),
        np.asarray(inputs["Wk"], np.float32),
        np.asarray(inputs["Wv"], np.float32),
        np.asarray(inputs["Wo"], np.float32),
        np.asarray(inputs["W_out_w"], np.float32),
        np.asarray(inputs["W_out_b"], np.float32),
        np.asarray(inputs["W_coord_w"], np.float32),
        np.asarray(inputs["W_coord_b"], np.float32),
    )
    track_tokens = np.asarray(inputs["track_tokens"], np.float32)
    feature_map = np.asarray(inputs["feature_map"], np.float32)
    feature_positions = np.asarray(inputs["feature_positions"], np.float32)
    spatial_bias = np.asarray(inputs["spatial_bias"], np.float32)

    in_maps = []
    for c in range(N_CORES):
        t, half = c // 2, c % 2
        qsl = slice(half * QH, (half + 1) * QH)
        m = dict(consts)
        m["tokT"] = _bf(track_tokens[t].T)
        m["posT"] = _f32(feature_positions[t, qsl].T)
        m["biasT"] = _bf(spatial_bias[t][:, qsl])
        m["fmapT"] = _f32(feature_map[t, qsl].T)
        in_maps.append(m)
    return in_maps


def kernel(
    track_tokens,
    feature_map,
    feature_positions,
    spatial_bias,
    Wq,
    Wk,
    Wv,
    Wo,
    W_out_w,
    W_out_b,
    W_coord_w,
    W_coord_b,
):
    global _NC_CACHE
    in_maps = _prep_in_maps(
        dict(
            track_tokens=track_tokens,
            feature_map=feature_map,
            feature_positions=feature_positions,
            spatial_bias=spatial_bias,
            Wq=Wq,
            Wk=Wk,
            Wv=Wv,
            Wo=Wo,
            W_out_w=W_out_w,
            W_out_b=W_out_b,
            W_coord_w=W_coord_w,
            W_coord_b=W_coord_b,
        )
    )

    if _NC_CACHE is None:
        _NC_CACHE = _build_nc()
    res = run_bass_kernel_spmd(_NC_CACHE, in_maps, core_ids=list(range(N_CORES)))

    out = np.empty((T, HW, DF), np.float32)
    for c in range(N_CORES):
        t, half = c // 2, c % 2
        qsl = slice(half * QH, (half + 1) * QH)
        out[t, qsl, :] = res.results[c]["out"].T
    return out



# revision 14
# speedup vs baseline: 1.3630x; 1.3630x over previous
"""AttentionalSplatting Trainium2 kernel (8 NeuronCores, SPMD).

Sharding: 8 cores = T(4) x HW-halves(2).  Core c handles t = c//2 and pixel
columns [ (c%2)*1152, (c%2+1)*1152 ).  Each core runs the full pipeline for
its (t, pixel-half): coord-proj + 2D RoPE -> Q/K/V proj -> qk-norm ->
scores(+spatial bias) -> softmax -> attend -> Wo -> W_out -> residual.
No cross-core communication is needed; outputs concatenate.

On-chip layout is feature-major ("transposed"): feature/head dims live on
SBUF partitions, pixels/tokens on the free dim.  Scores are computed as
S^T (m on partitions, q free) so the attend matmul consumes the softmax
numerator directly and softmax sums arrive free via a ones-column appended
to V.

The per-element softmax work (the (M x HW) score tensor per head) is split
across TWO engines so it runs concurrently with the PE:
 - token chunks 0..511 ("ACT path"): spatial bias is injected into PSUM via
   identity matmuls and the Scalar engine computes exp(S + B) exactly.
 - token chunks 512..1023 ("DVE path"): since Q,K are L2-normalized,
   |S| <= 1/sqrt(32) = 0.177, so exp(S+B) = exp(B)*exp(S) ~= exp(B)*(1+S)
   to ~1.5%.  exp(B) is precomputed on host; the Vector engine computes
   (1+S)*exp(B) in a single scalar_tensor_tensor op.  The shared softmax
   denominator keeps the two halves consistent.
"""

import math
import sys

import numpy as np

sys.path.insert(0, "/opt/trn_rl_repo")

import ml_dtypes  # noqa: E402

import concourse.bass as bass  # noqa: E402
import concourse.bacc as bacc  # noqa: E402
import concourse.tile as tile  # noqa: E402
from concourse import mybir  # noqa: E402
from concourse.bass_utils import run_bass_kernel_spmd  # noqa: E402

T, M, HW, DF, H = 4, 1024, 2304, 256, 8
DKH = DF // H  # 32
QH = HW // 2  # 1152 pixels per core
SCALE = 1.0 / math.sqrt(DKH)
D_HALF = DF // 2  # 128
D_QUART = DF // 4  # 64
THETA = (100.0 ** (-2.0 * np.arange(D_QUART, dtype=np.float32) / D_HALF)).astype(
    np.float32
)

F32 = mybir.dt.float32
BF16 = mybir.dt.bfloat16
AF = mybir.ActivationFunctionType
ALU = mybir.AluOpType
BF16NP = ml_dtypes.bfloat16

N_CORES = 8
Q_BLOCKS = [(0, 512), (512, 512), (1024, 128)]
# token chunks 0-3 -> exact exp on Scalar engine (bias injected via PE)
# token chunks 4-7 -> (1+S)*exp(B) on Vector engine
M_TRIPS = [(0, 1), (2, 3), (4, 5), (6, 7)]
ACT_TRIPS = {0, 1}
N_ACT_MC = 4  # token chunks on the ACT path (rows 0 .. N_ACT_MC*128)


def _bf(x):
    return np.ascontiguousarray(np.asarray(x, np.float32)).astype(BF16NP)


def _f32(x):
    return np.ascontiguousarray(np.asarray(x, np.float32))


def _host_constants(Wq, Wk, Wv, Wo, W_out_w, W_out_b, W_coord_w, W_coord_b):
    """Shared (core-independent) device constants, host-precomputed."""
    # pair-swapped coord weights for RoPE (swap even/odd output columns)
    perm = np.arange(DF)
    perm = perm.reshape(-1, 2)[:, ::-1].reshape(-1)
    wcsw = W_coord_w[:, perm]
    wcbsw = W_coord_b[perm]
    # signed duplicated theta: [-t0, +t0, -t1, +t1, ...]
    th = np.empty((1, D_HALF), np.float32)
    th[0, 0::2] = -THETA
    th[0, 1::2] = THETA
    # block-ones for per-head sum of squares: dtile k maps its 128 feature
    # rows onto head columns 4k..4k+3
    bones = np.zeros((2, 128, 8), np.float32)
    for k in range(2):
        for d in range(128):
            bones[k, d, 4 * k + d // 32] = 1.0
    # expand per-head scalars (8, q) back to the 128 feature rows of dtile k
    exp8 = np.zeros((2, 8, 128), np.float32)
    for k in range(2):
        for d in range(128):
            exp8[k, 4 * k + d // 32, d] = 1.0
    # expand per-head inv-sum (8, q) to paired attend-output rows:
    # pair j holds head 2j at rows 1..33 and head 2j+1 at rows 65..97
    expP = np.zeros((4, 8, 128), np.float32)
    for j in range(4):
        expP[j, 2 * j, 1:33] = 1.0
        expP[j, 2 * j + 1, 65:97] = 1.0
    # Wo rearranged to the paired attend-output row layout (sumexp rows = 0)
    wo_aug = np.zeros((4, 128, DF), np.float32)
    for j in range(4):
        wo_aug[j, 1:33, :] = Wo[(2 * j) * 32 : (2 * j + 1) * 32, :]
        wo_aug[j, 65:97, :] = Wo[(2 * j + 1) * 32 : (2 * j + 2) * 32, :]
    return {
        "wq": _bf(Wq),
        "wk": _bf(Wk),
        "wv": _bf(Wv),
        "wo_aug": _bf(wo_aug),
        "wout": _bf(W_out_w),
        "woutb": _f32(W_out_b).reshape(DF, 1),
        "wc": _f32(np.vstack([W_coord_w, W_coord_b[None, :]])),
        "wcsw": _f32(np.vstack([wcsw, wcbsw[None, :]])),
        "theta2s": th,
        "bones": _bf(bones),
        "exp8": exp8,
        "expP": expP,
        "ident": _bf(np.eye(128, dtype=np.float32)),
    }


_NC_CACHE = None


def _build_nc():
    nc = bacc.Bacc(
        "TRN2",
        target_bir_lowering=False,
        debug=False,
        enable_asserts=True,
        num_devices=N_CORES,
    )
    d = {}
    inp = lambda name, shape, dt: d.__setitem__(
        name, nc.declare_dram_parameter(name, list(shape), dt, isOutput=False)
    )
    inp("tokT", (DF, M), BF16)
    inp("posT", (3, QH), F32)
    inp("biasT", (N_ACT_MC * 128, QH), BF16)  # raw bias, token rows 0..511
    inp("ebT", ((8 - N_ACT_MC) * 128, QH), BF16)  # exp(bias), rows 512..1023
    inp("fmapT", (DF, QH), F32)
    inp("wq", (DF, DF), BF16)
    inp("wk", (DF, DF), BF16)
    inp("wv", (DF, DF), BF16)
    inp("wo_aug", (4, 128, DF), BF16)
    inp("wout", (DF, DF), BF16)
    inp("woutb", (DF, 1), F32)
    inp("wc", (3, DF), F32)
    inp("wcsw", (3, DF), F32)
    inp("theta2s", (1, D_HALF), F32)
    inp("bones", (2, 128, 8), BF16)
    inp("exp8", (2, 8, 128), F32)
    inp("expP", (4, 8, 128), F32)
    inp("ident", (128, 128), BF16)
    out = nc.declare_dram_parameter("out", [DF, QH], F32, isOutput=True)

    import os as _os

    with tile.TileContext(
        nc, trace_sim=bool(_os.environ.get("KERNEL_TRACE_SIM"))
    ) as tc:
        _body(nc, tc, d, out)
    nc.compile()
    return nc


def _body(nc, tc, d, out_dram):
    mm = nc.tensor.matmul
    act = nc.scalar.activation
    dma = nc.sync.dma_start

    with (
        tc.tile_pool(name="const", bufs=1) as cpool,
        tc.tile_pool(name="work", bufs=1) as wpool,
        tc.tile_pool(name="persist", bufs=1) as ppool,
        tc.tile_pool(name="epool", bufs=6) as epool,
        # 3 x 2 banks for score tiles / prologue projections
        tc.tile_pool(name="psS", bufs=3, space=bass.MemorySpace.PSUM) as psS,
        # 2 x 1 bank for attend accumulators / small matmuls
        tc.tile_pool(name="psO", bufs=2, space=bass.MemorySpace.PSUM) as psO,
    ):
        # ---- load constants / inputs to SBUF ----
        # 256-row tensors fold to (128, 2, ...): [:, kt, ...] = rows kt*128..
        def load(name, shape, dt, rearrange=None, **kw):
            t = cpool.tile(list(shape), dt, tag=name)
            src = d[name][:]
            if rearrange is not None:
                src = src.rearrange(rearrange, **kw)
            dma(t[:], src)
            return t

        fold = "(k p) d -> p k d"
        wq = load("wq", (128, 2, DF), BF16, fold, p=128)
        wk = load("wk", (128, 2, DF), BF16, fold, p=128)
        wv = load("wv", (128, 2, DF), BF16, fold, p=128)
        wo_aug = load("wo_aug", (128, 4, DF), BF16, "j p d -> p j d")
        wout = load("wout", (128, 2, DF), BF16, fold, p=128)
        woutb = load("woutb", (128, 2, 1), F32, fold, p=128)
        wc = load("wc", (3, DF), F32)
        wcsw = load("wcsw", (3, DF), F32)
        th2 = load("theta2s", (1, D_HALF), F32)
        bones = load("bones", (128, 2, 8), BF16, "k p h -> p k h")
        exp8 = load("exp8", (8, 2, 128), F32, "k h d -> h k d")
        expP = load("expP", (8, 4, 128), F32, "j s e -> s j e")
        ident = load("ident", (128, 128), BF16)
        posT = load("posT", (3, QH), F32)
        posT2 = cpool.tile([1, 2, QH], F32, tag="posT2")
        dma(posT2[:], d["posT"][0:2, :].rearrange("(o a) q -> o a q", o=1))
        fmapT = load("fmapT", (128, 2, QH), F32, fold, p=128)
        tokT = load("tokT", (128, 2, M), BF16, fold, p=128)
        # bias (ACT half) and exp(bias) (DVE half), mc on a middle free dim
        bias_t = load("biasT", (128, N_ACT_MC, QH), BF16, "(c p) q -> p c q", p=128)
        eb_t = load("ebT", (128, 8 - N_ACT_MC, QH), BF16, "(c p) q -> p c q", p=128)

        def const_tile(val, name):
            t = cpool.tile([128, 1], F32, tag=name)
            nc.vector.memset(t[:], val)
            return t

        halfpi = const_tile(math.pi / 2.0, "halfpi")
        zero_c = const_tile(0.0, "zeroc")
        lnscale = const_tile(math.log(SCALE), "lnscale")

        # prologue psum helper: chunks of <=1024 columns from the psS pool,
        # matmuls emitted in <=512-column blocks (one PSUM bank each).
        def chunks(n):
            out = []
            o = 0
            while o < n:
                w = min(1024, n - o)
                subs = []
                so = 0
                while so < w:
                    sw = min(512, w - so)
                    subs.append((so, sw))
                    so += sw
                out.append((o, w, subs))
                o += w
            return out

        # ---- Q path: Qin^T (and pair-swapped) = Wc^T @ pos^T (+b) ----
        qin = []
        for dt_i, w in enumerate((wc, wcsw)):
            for half in range(2):
                t = wpool.tile([128, QH], BF16, tag=f"qin{dt_i}{half}")
                for co, cw, subs in chunks(QH):
                    ps = psS.tile([128, 2, 512], F32, tag="s")
                    psf = ps[:].rearrange("p c q -> p (c q)")
                    for so, sw in subs:
                        mm(
                            psf[:, so : so + sw],
                            w[:, half * 128 : (half + 1) * 128],
                            posT[:, co + so : co + so + sw],
                        )
                    nc.vector.tensor_copy(t[:, co : co + cw], psf[:, 0:cw])
                qin.append(t)
        qin0, qin1, qins0, qins1 = qin

        # ---- RoPE tables: one angle matmul per axis (signed theta) ----
        cs = []
        for axis in range(2):
            c_t = wpool.tile([128, QH], BF16, tag=f"cos{axis}")
            s_t = wpool.tile([128, QH], BF16, tag=f"sin{axis}")
            for co, cw, subs in chunks(QH):
                ps = psS.tile([128, 2, 512], F32, tag="s")
                psf = ps[:].rearrange("p c q -> p (c q)")
                for so, sw in subs:
                    mm(
                        psf[:, so : so + sw],
                        th2[:, :],
                        posT2[:, axis, co + so : co + so + sw],
                    )
                act(c_t[:, co : co + cw], psf[:, 0:cw], AF.Sin, bias=halfpi[:])
                act(s_t[:, co : co + cw], psf[:, 0:cw], AF.Sin)
            cs.append((c_t, s_t))

        roped = []
        for dt_i, (q, qs) in enumerate(((qin0, qins0), (qin1, qins1))):
            c_t, s_t = cs[dt_i]
            t1 = wpool.tile([128, QH], BF16, tag=f"ropea{dt_i}")
            nc.vector.tensor_mul(t1[:], q[:], c_t[:])
            t2 = wpool.tile([128, QH], BF16, tag=f"ropeb{dt_i}")
            nc.vector.tensor_mul(t2[:], qs[:], s_t[:])
            r = wpool.tile([128, QH], BF16, tag=f"roped{dt_i}")
            nc.vector.tensor_add(r[:], t1[:], t2[:])
            roped.append(r)

        # ---- projection + qk-norm, processed in <=1024-column chunks ----
        def proj_norm(w_sb, rhs_tiles, n, ln_bias, name):
            """normalized bf16 (128, n) x2 head-major tiles of W^T @ rhs"""
            outs = [
                ppool.tile([128, n], BF16, tag=f"{name}T{dt_i}", name=f"{name}T{dt_i}")
                for dt_i in range(2)
            ]
            for co, cw, subs in chunks(n):
                ps_l = []
                sq_ps = psS.tile([8, 1024], F32, tag="s")
                bfl = []
                for dt_i in range(2):
                    ps = psS.tile([128, 2, 512], F32, tag="s")
                    psf = ps[:].rearrange("p c q -> p (c q)")
                    for so, sw in subs:
                        for kt in range(2):
                            mm(
                                psf[:, so : so + sw],
                                w_sb[:, kt, dt_i * 128 : (dt_i + 1) * 128],
                                rhs_tiles[kt][:, co + so : co + so + sw],
                                start=(kt == 0),
                                stop=(kt == 1),
                            )
                    tb = epool.tile([128, 2, 512], BF16, tag="E")
                    tbf = tb[:].rearrange("p c q -> p (c q)")
                    nc.vector.tensor_copy(tbf[:, 0:cw], psf[:, 0:cw])
                    bfl.append(tb)
                    sq = epool.tile([128, 2, 512], BF16, tag="E")
                    sqf = sq[:].rearrange("p c q -> p (c q)")
                    nc.vector.tensor_mul(sqf[:, 0:cw], tbf[:, 0:cw], tbf[:, 0:cw])
                    for so, sw in subs:
                        mm(
                            sq_ps[:, so : so + sw],
                            bones[:, dt_i, :],
                            sqf[:, so : so + sw],
                            start=(dt_i == 0),
                            stop=(dt_i == 1),
                        )
                    ps_l.append(ps)
                lnt = wpool.tile([8, 1024], F32, tag=f"{name}ln", bufs=2)
                act(lnt[:, 0:cw], sq_ps[:, 0:cw], AF.Ln)
                if ln_bias is None:
                    ln_bias = zero_c
                invn = wpool.tile([8, 1024], F32, tag=f"{name}inv", bufs=2)
                act(invn[:, 0:cw], lnt[:, 0:cw], AF.Exp, scale=-0.5, bias=ln_bias[:8, :])
                for dt_i in range(2):
                    psx = psS.tile([128, 2, 512], F32, tag="s")
                    psxf = psx[:].rearrange("p c q -> p (c q)")
                    for so, sw in subs:
                        mm(
                            psxf[:, so : so + sw],
                            exp8[:, dt_i, :],
                            invn[:, so : so + sw],
                        )
                    tbf = bfl[dt_i][:].rearrange("p c q -> p (c q)")
                    nc.vector.tensor_mul(
                        outs[dt_i][:, co : co + cw], tbf[:, 0:cw], psxf[:, 0:cw]
                    )
            return outs

        qnT = proj_norm(wq, roped, QH, lnscale, "q")
        tok_tiles = [tokT[:, 0, :], tokT[:, 1, :]]
        knT = proj_norm(wk, tok_tiles, M, None, "k")

        # ---- V (token-major) with ones column:  vsb[mc] = (128, 8, 33) ----
        vsb = []
        for mc in range(8):
            ps = psO.tile([128, 256], F32, tag="o")
            for kt in range(2):
                mm(
                    ps[:],
                    tokT[:, kt, mc * 128 : (mc + 1) * 128],
                    wv[:, kt, :],
                    start=(kt == 0),
                    stop=(kt == 1),
                )
            vt = ppool.tile([128, 8, 33], BF16, tag=f"v{mc}")
            nc.vector.memset(vt[:, :, 0:1], 1.0)
            nc.vector.tensor_copy(
                vt[:, :, 1:33], ps[:].rearrange("p (h e) -> p h e", h=8)
            )
            vsb.append(vt)

        # ---- main attention loop ----
        # pair j: head 2j accumulates at psum rows 0..32, head 2j+1 at 64..96
        osb = []  # per pair (128, QH) bf16, rows 0/64 = sumexp
        for j in range(4):
            t = ppool.tile([128, QH], BF16, tag=f"osb{j}")
            osb.append(t)

        for qo, qb in Q_BLOCKS:
            for j in range(4):
                heads = (2 * j, 2 * j + 1)
                dt_i = j // 2
                o_ps = psO.tile([128, 512], F32, tag="o")
                for ti, trip in enumerate(M_TRIPS):
                    is_act = ti in ACT_TRIPS
                    e_ts = {}
                    s_tiles = {}
                    for h in heads:
                        s_ps = psS.tile([128, 2, 512], F32, tag="s")
                        s_tiles[h] = s_ps
                        if is_act:
                            for i, mc in enumerate(trip):
                                mm(
                                    s_ps[:, i, 0:qb],
                                    ident[:],
                                    bias_t[:, mc, qo : qo + qb],
                                    start=True,
                                    stop=False,
                                )
                    for i, mc in enumerate(trip):
                        for h in heads:
                            hp = (h % 4) * 32
                            mm(
                                s_tiles[h][:, i, 0:qb],
                                knT[dt_i][hp : hp + 32, mc * 128 : (mc + 1) * 128],
                                qnT[dt_i][hp : hp + 32, qo : qo + qb],
                                start=(not is_act),
                                stop=True,
                                tile_position=(hp, 0),
                            )
                    for h in heads:
                        e_t = epool.tile([128, 2, 512], BF16, tag="E")
                        if is_act:
                            act(e_t[:, :, 0:qb], s_tiles[h][:, :, 0:qb], AF.Exp)
                        else:
                            nc.vector.scalar_tensor_tensor(
                                e_t[:, :, 0:qb],
                                s_tiles[h][:, :, 0:qb],
                                1.0,
                                eb_t[:, trip[0] - N_ACT_MC : trip[0] - N_ACT_MC + 2, qo : qo + qb],
                                ALU.add,
                                ALU.mult,
                            )
                        e_ts[h] = e_t
                    for i, mc in enumerate(trip):
                        for h in heads:
                            base = 64 * (h % 2)
                            mm(
                                o_ps[base : base + 33, 0:qb],
                                vsb[mc][:, h, :],
                                e_ts[h][:, i, 0:qb],
                                start=(mc == 0),
                                stop=(mc == 7),
                                tile_position=(0, base),
                            )
                nc.scalar.copy(osb[j][:, qo : qo + qb], o_ps[:, 0:qb])

        # ---- softmax denominators: gather row 0 of each head, invert ----
        sumE = wpool.tile([8, QH], BF16, tag="sumE")
        for h in range(8):
            j, r = h // 2, 64 * (h % 2)
            dma(sumE[h : h + 1, :], osb[h // 2][r : r + 1, :])
        lnS = wpool.tile([8, QH], F32, tag="lnS")
        act(lnS[:], sumE[:], AF.Ln)
        invS = wpool.tile([8, QH], F32, tag="invS")
        act(invS[:], lnS[:], AF.Exp, scale=-1.0)

        for j in range(4):
            for qo, qb in Q_BLOCKS:
                ps = psO.tile([128, 512], F32, tag="o")
                mm(ps[:, 0:qb], expP[:, j, :], invS[:, qo : qo + qb])
                nc.vector.tensor_mul(
                    osb[j][:, qo : qo + qb], osb[j][:, qo : qo + qb], ps[:, 0:qb]
                )

        # ---- output projections + residual ----
        o1b = [
            wpool.tile([128, QH], BF16, tag=f"o1b{dt_i}", name=f"o1b{dt_i}")
            for dt_i in range(2)
        ]
        for dt_i in range(2):
            for co, cw, subs in chunks(QH):
                ps = psS.tile([128, 2, 512], F32, tag="s")
                psf = ps[:].rearrange("p c q -> p (c q)")
                for so, sw in subs:
                    for j in range(4):
                        mm(
                            psf[:, so : so + sw],
                            wo_aug[:, j, dt_i * 128 : (dt_i + 1) * 128],
                            osb[j][:, co + so : co + so + sw],
                            start=(j == 0),
                            stop=(j == 3),
                        )
                nc.vector.tensor_copy(o1b[dt_i][:, co : co + cw], psf[:, 0:cw])

        for dt_i in range(2):
            for co, cw, subs in chunks(QH):
                ps = psS.tile([128, 2, 512], F32, tag="s")
                psf = ps[:].rearrange("p c q -> p (c q)")
                for so, sw in subs:
                    for kt in range(2):
                        mm(
                            psf[:, so : so + sw],
                            wout[:, kt, dt_i * 128 : (dt_i + 1) * 128],
                            o1b[kt][:, co + so : co + so + sw],
                            start=(kt == 0),
                            stop=(kt == 1),
                        )
                r1 = wpool.tile([128, 1024], F32, tag=f"res{dt_i}{co}")
                nc.vector.scalar_tensor_tensor(
                    r1[:, 0:cw],
                    psf[:, 0:cw],
                    woutb[:, dt_i, :],
                    fmapT[:, dt_i, co : co + cw],
                    ALU.add,
                    ALU.add,
                )
                dma(
                    out_dram[dt_i * 128 : (dt_i + 1) * 128, co : co + cw],
                    r1[:, 0:cw],
                )


def _prep_in_maps(inputs):
    consts = _host_constants(
        np.asarray(inputs["Wq"], np.float32),
        np.asarray(inputs["Wk"], np.float32),
        np.asarray(inputs["Wv"], np.float32),
        np.asarray(inputs["Wo"], np.float32),
        np.asarray(inputs["W_out_w"], np.float32),
        np.asarray(inputs["W_out_b"], np.float32),
        np.asarray(inputs["W_coord_w"], np.float32),
        np.asarray(inputs["W_coord_b"], np.float32),
    )
    track_tokens = np.asarray(inputs["track_tokens"], np.float32)
    feature_map = np.asarray(inputs["feature_map"], np.float32)
    feature_positions = np.asarray(inputs["feature_positions"], np.float32)
    spatial_bias = np.asarray(inputs["spatial_bias"], np.float32)

    msplit = N_ACT_MC * 128
    in_maps = []
    for c in range(N_CORES):
        t, half = c // 2, c % 2
        qsl = slice(half * QH, (half + 1) * QH)
        m = dict(consts)
        m["tokT"] = _bf(track_tokens[t].T)
        pos = feature_positions[t, qsl].T
        m["posT"] = _f32(np.vstack([pos, np.ones((1, QH), np.float32)]))
        bias_cols = spatial_bias[t][:, qsl]
        m["biasT"] = _bf(bias_cols[:msplit])
        m["ebT"] = _bf(np.exp(bias_cols[msplit:]))
        m["fmapT"] = _f32(feature_map[t, qsl].T)
        in_maps.append(m)
    return in_maps


def kernel(
    track_tokens,
    feature_map,
    feature_positions,
    spatial_bias,
    Wq,
    Wk,
    Wv,
    Wo,
    W_out_w,
    W_out_b,
    W_coord_w,
    W_coord_b,
):
    global _NC_CACHE
    in_maps = _prep_in_maps(
        dict(
            track_tokens=track_tokens,
            feature_map=feature_map,
            feature_positions=feature_positions,
            spatial_bias=spatial_bias,
            Wq=Wq,
            Wk=Wk,
            Wv=Wv,
            Wo=Wo,
            W_out_w=W_out_w,
            W_out_b=W_out_b,
            W_coord_w=W_coord_w,
            W_coord_b=W_coord_b,
        )
    )

    if _NC_CACHE is None:
        _NC_CACHE = _build_nc()
    res = run_bass_kernel_spmd(_NC_CACHE, in_maps, core_ids=list(range(N_CORES)))

    out = np.empty((T, HW, DF), np.float32)
    for c in range(N_CORES):
        t, half = c // 2, c % 2
        qsl = slice(half * QH, (half + 1) * QH)
        out[t, qsl, :] = res.results[c]["out"].T
    return out


# revision 17
# speedup vs baseline: 1.5346x; 1.1259x over previous
"""AttentionalSplatting Trainium2 kernel (8 NeuronCores, SPMD).

Sharding: 8 cores = T(4) x HW-halves(2).  Core c handles t = c//2 and pixel
columns [ (c%2)*1152, (c%2+1)*1152 ).  Each core runs the full pipeline for
its (t, pixel-half): coord-proj + 2D RoPE -> Q/K/V proj -> qk-norm ->
scores(+spatial bias) -> softmax -> attend -> Wo -> W_out -> residual.
No cross-core communication is needed; outputs concatenate.

On-chip layout is feature-major ("transposed"): feature/head dims live on
SBUF partitions, pixels/tokens on the free dim.  Scores are computed as
S^T (m on partitions, q free) so the attend matmul consumes the softmax
numerator directly and softmax sums arrive free via a ones-column appended
to V.

The per-element softmax work (the (M x HW) score tensor per head) is split
across TWO engines so it runs concurrently with the PE:
 - token chunks 0..511 ("ACT path"): spatial bias is injected into PSUM via
   identity matmuls and the Scalar engine computes exp(S + B) exactly.
 - token chunks 512..1023 ("DVE path"): since Q,K are L2-normalized,
   |S| <= 1/sqrt(32) = 0.177, so exp(S+B) = exp(B)*exp(S) ~= exp(B)*(1+S)
   to ~1.5%.  exp(B) is precomputed on host; the Vector engine computes
   (1+S)*exp(B) in a single scalar_tensor_tensor op.  The shared softmax
   denominator keeps the two halves consistent.
"""

import math
import sys

import numpy as np

sys.path.insert(0, "/opt/trn_rl_repo")

import ml_dtypes  # noqa: E402

import concourse.bass as bass  # noqa: E402
import concourse.bacc as bacc  # noqa: E402
import concourse.tile as tile  # noqa: E402
from concourse import mybir  # noqa: E402
from concourse.bass_utils import run_bass_kernel_spmd  # noqa: E402

T, M, HW, DF, H = 4, 1024, 2304, 256, 8
DKH = DF // H  # 32
QH = HW // 2  # 1152 pixels per core
SCALE = 1.0 / math.sqrt(DKH)
D_HALF = DF // 2  # 128
D_QUART = DF // 4  # 64
THETA = (100.0 ** (-2.0 * np.arange(D_QUART, dtype=np.float32) / D_HALF)).astype(
    np.float32
)

F32 = mybir.dt.float32
BF16 = mybir.dt.bfloat16
AF = mybir.ActivationFunctionType
ALU = mybir.AluOpType
BF16NP = ml_dtypes.bfloat16

N_CORES = 8
Q_BLOCKS = [(0, 512), (512, 512), (1024, 128)]
# token chunks 0-3 -> exact exp on Scalar engine (bias injected via PE)
# token chunks 4-7 -> (1+S)*exp(B) on Vector engine
M_TRIPS = [(0, 1), (2, 3), (4, 5), (6, 7)]
ACT_TRIPS = {0, 1}
N_ACT_MC = 4  # token chunks on the ACT path (rows 0 .. N_ACT_MC*128)


def _bf(x):
    return np.ascontiguousarray(np.asarray(x, np.float32)).astype(BF16NP)


def _f32(x):
    return np.ascontiguousarray(np.asarray(x, np.float32))


def _rope2d_host(feat, pos):
    """reference _rope_2d in numpy: feat (N, DF), pos (N, 2)"""
    def rope1(f, p):
        fr = f.reshape(f.shape[0], D_QUART, 2)
        ang = p[:, None] * THETA[None, :]
        c, s = np.cos(ang), np.sin(ang)
        ev = fr[..., 0] * c - fr[..., 1] * s
        od = fr[..., 0] * s + fr[..., 1] * c
        return np.stack([ev, od], axis=-1).reshape(f.shape)
    xr = rope1(feat[:, :D_HALF], pos[:, 0])
    yr = rope1(feat[:, D_HALF:], pos[:, 1])
    return np.concatenate([xr, yr], axis=-1)


def _host_constants(Wq, Wk, Wv, Wo, W_out_w, W_out_b, W_coord_w, W_coord_b):
    """Shared (core-independent) device constants, host-precomputed."""
    # block-ones for per-head sum of squares: dtile k maps its 128 feature
    # rows onto head columns 4k..4k+3
    bones = np.zeros((2, 128, 8), np.float32)
    for k in range(2):
        for d in range(128):
            bones[k, d, 4 * k + d // 32] = 1.0
    # expand per-head scalars (8, q) back to the 128 feature rows of dtile k
    exp8 = np.zeros((2, 8, 128), np.float32)
    for k in range(2):
        for d in range(128):
            exp8[k, 4 * k + d // 32, d] = 1.0
    # expand per-head inv-sum (8, q) to paired attend-output rows:
    # pair j holds head 2j at rows 1..33 and head 2j+1 at rows 65..97
    expP = np.zeros((4, 8, 128), np.float32)
    for j in range(4):
        expP[j, 2 * j, 1:33] = 1.0
        expP[j, 2 * j + 1, 65:97] = 1.0
    # Wo rearranged to the paired attend-output row layout (sumexp rows = 0)
    wo_aug = np.zeros((4, 128, DF), np.float32)
    for j in range(4):
        wo_aug[j, 1:33, :] = Wo[(2 * j) * 32 : (2 * j + 1) * 32, :]
        wo_aug[j, 65:97, :] = Wo[(2 * j + 1) * 32 : (2 * j + 2) * 32, :]
    return {
        "wq": _bf(Wq),
        "wk": _bf(Wk),
        "wv": _bf(Wv),
        "wo_aug": _bf(wo_aug),
        "wout": _bf(W_out_w),
        "woutb": _f32(W_out_b).reshape(DF, 1),
        "bones": _bf(bones),
        "exp8": exp8,
        "expP": expP,
        "ident": _bf(np.eye(128, dtype=np.float32)),
    }


_NC_CACHE = None


def _build_nc():
    nc = bacc.Bacc(
        "TRN2",
        target_bir_lowering=False,
        debug=False,
        enable_asserts=True,
        num_devices=N_CORES,
    )
    d = {}
    inp = lambda name, shape, dt: d.__setitem__(
        name, nc.declare_dram_parameter(name, list(shape), dt, isOutput=False)
    )
    inp("tokT", (DF, M), BF16)
    inp("ropedT", (DF, QH), BF16)  # host: rope_2d(pos @ Wc + b, pos), transposed
    inp("biasT", (N_ACT_MC * 128, QH), BF16)  # raw bias, token rows 0..511
    inp("ebT", ((8 - N_ACT_MC) * 128, QH), BF16)  # exp(bias), rows 512..1023
    inp("fmapT", (DF, QH), F32)
    inp("wq", (DF, DF), BF16)
    inp("wk", (DF, DF), BF16)
    inp("wv", (DF, DF), BF16)
    inp("wo_aug", (4, 128, DF), BF16)
    inp("wout", (DF, DF), BF16)
    inp("woutb", (DF, 1), F32)
    inp("bones", (2, 128, 8), BF16)
    inp("exp8", (2, 8, 128), F32)
    inp("expP", (4, 8, 128), F32)
    inp("ident", (128, 128), BF16)
    out = nc.declare_dram_parameter("out", [DF, QH], F32, isOutput=True)

    import os as _os

    with tile.TileContext(
        nc, trace_sim=bool(_os.environ.get("KERNEL_TRACE_SIM"))
    ) as tc:
        _body(nc, tc, d, out)
    nc.compile()
    return nc


def _body(nc, tc, d, out_dram):
    mm = nc.tensor.matmul
    act = nc.scalar.activation
    dma = nc.sync.dma_start

    with (
        tc.tile_pool(name="const", bufs=1) as cpool,
        tc.tile_pool(name="work", bufs=1) as wpool,
        tc.tile_pool(name="persist", bufs=1) as ppool,
        tc.tile_pool(name="epool", bufs=6) as epool,
        # 3 x 2 banks for score tiles / prologue projections
        tc.tile_pool(name="psS", bufs=3, space=bass.MemorySpace.PSUM) as psS,
        # 2 x 1 bank for attend accumulators / small matmuls
        tc.tile_pool(name="psO", bufs=2, space=bass.MemorySpace.PSUM) as psO,
    ):
        # ---- load constants / inputs to SBUF ----
        # 256-row tensors fold to (128, 2, ...): [:, kt, ...] = rows kt*128..
        def load(name, shape, dt, rearrange=None, **kw):
            t = cpool.tile(list(shape), dt, tag=name)
            src = d[name][:]
            if rearrange is not None:
                src = src.rearrange(rearrange, **kw)
            dma(t[:], src)
            return t

        fold = "(k p) d -> p k d"
        wq = load("wq", (128, 2, DF), BF16, fold, p=128)
        wk = load("wk", (128, 2, DF), BF16, fold, p=128)
        wv = load("wv", (128, 2, DF), BF16, fold, p=128)
        wo_aug = load("wo_aug", (128, 4, DF), BF16, "j p d -> p j d")
        wout = load("wout", (128, 2, DF), BF16, fold, p=128)
        woutb = load("woutb", (128, 2, 1), F32, fold, p=128)
        bones = load("bones", (128, 2, 8), BF16, "k p h -> p k h")
        exp8 = load("exp8", (8, 2, 128), F32, "k h d -> h k d")
        expP = load("expP", (8, 4, 128), F32, "j s e -> s j e")
        ident = load("ident", (128, 128), BF16)
        tokT = load("tokT", (128, 2, M), BF16, fold, p=128)
        ropedT = load("ropedT", (128, 2, QH), BF16, fold, p=128)
        # bias (ACT half) and exp(bias) (DVE half), mc on a middle free dim
        bias_t = load("biasT", (128, N_ACT_MC, QH), BF16, "(c p) q -> p c q", p=128)
        eb_t = load("ebT", (128, 8 - N_ACT_MC, QH), BF16, "(c p) q -> p c q", p=128)
        fmapT = load("fmapT", (128, 2, QH), F32, fold, p=128)

        def const_tile(val, name):
            t = cpool.tile([128, 1], F32, tag=name)
            nc.vector.memset(t[:], val)
            return t

        zero_c = const_tile(0.0, "zeroc")
        lnscale = const_tile(math.log(SCALE), "lnscale")

        # prologue psum helper: chunks of <=1024 columns from the psS pool,
        # matmuls emitted in <=512-column blocks (one PSUM bank each).
        def chunks(n):
            out = []
            o = 0
            while o < n:
                w = min(1024, n - o)
                subs = []
                so = 0
                while so < w:
                    sw = min(512, w - so)
                    subs.append((so, sw))
                    so += sw
                out.append((o, w, subs))
                o += w
            return out

        # ---- projection + qk-norm, processed in <=1024-column chunks ----
        def proj_norm(w_sb, rhs_tiles, n, ln_bias, name):
            """normalized bf16 (128, n) x2 head-major tiles of W^T @ rhs"""
            outs = [
                ppool.tile([128, n], BF16, tag=f"{name}T{dt_i}", name=f"{name}T{dt_i}")
                for dt_i in range(2)
            ]
            for co, cw, subs in chunks(n):
                ps_l = []
                sq_ps = psS.tile([8, 1024], F32, tag="s")
                bfl = []
                for dt_i in range(2):
                    ps = psS.tile([128, 2, 512], F32, tag="s")
                    psf = ps[:].rearrange("p c q -> p (c q)")
                    for so, sw in subs:
                        for kt in range(2):
                            mm(
                                psf[:, so : so + sw],
                                w_sb[:, kt, dt_i * 128 : (dt_i + 1) * 128],
                                rhs_tiles[kt][:, co + so : co + so + sw],
                                start=(kt == 0),
                                stop=(kt == 1),
                            )
                    tb = epool.tile([128, 2, 512], BF16, tag="E")
                    tbf = tb[:].rearrange("p c q -> p (c q)")
                    nc.vector.tensor_copy(tbf[:, 0:cw], psf[:, 0:cw])
                    bfl.append(tb)
                    sq = epool.tile([128, 2, 512], BF16, tag="E")
                    sqf = sq[:].rearrange("p c q -> p (c q)")
                    nc.vector.tensor_mul(sqf[:, 0:cw], tbf[:, 0:cw], tbf[:, 0:cw])
                    for so, sw in subs:
                        mm(
                            sq_ps[:, so : so + sw],
                            bones[:, dt_i, :],
                            sqf[:, so : so + sw],
                            start=(dt_i == 0),
                            stop=(dt_i == 1),
                        )
                    ps_l.append(ps)
                lnt = wpool.tile([8, 1024], F32, tag=f"{name}ln", bufs=2)
                act(lnt[:, 0:cw], sq_ps[:, 0:cw], AF.Ln)
                if ln_bias is None:
                    ln_bias = zero_c
                invn = wpool.tile([8, 1024], F32, tag=f"{name}inv", bufs=2)
                act(invn[:, 0:cw], lnt[:, 0:cw], AF.Exp, scale=-0.5, bias=ln_bias[:8, :])
                for dt_i in range(2):
                    psx = psS.tile([128, 2, 512], F32, tag="s")
                    psxf = psx[:].rearrange("p c q -> p (c q)")
                    for so, sw in subs:
                        mm(
                            psxf[:, so : so + sw],
                            exp8[:, dt_i, :],
                            invn[:, so : so + sw],
                        )
                    tbf = bfl[dt_i][:].rearrange("p c q -> p (c q)")
                    nc.vector.tensor_mul(
                        outs[dt_i][:, co : co + cw], tbf[:, 0:cw], psxf[:, 0:cw]
                    )
            return outs

        qnT = proj_norm(wq, [ropedT[:, 0, :], ropedT[:, 1, :]], QH, lnscale, "q")
        tok_tiles = [tokT[:, 0, :], tokT[:, 1, :]]
        knT = proj_norm(wk, tok_tiles, M, None, "k")

        # ---- V (token-major) with ones column:  vsb[mc] = (128, 8, 33) ----
        vsb = []
        for mc in range(8):
            ps = psO.tile([128, 256], F32, tag="o")
            for kt in range(2):
                mm(
                    ps[:],
                    tokT[:, kt, mc * 128 : (mc + 1) * 128],
                    wv[:, kt, :],
                    start=(kt == 0),
                    stop=(kt == 1),
                )
            vt = ppool.tile([128, 8, 33], BF16, tag=f"v{mc}")
            nc.vector.memset(vt[:, :, 0:1], 1.0)
            nc.vector.tensor_copy(
                vt[:, :, 1:33], ps[:].rearrange("p (h e) -> p h e", h=8)
            )
            vsb.append(vt)

        # ---- main attention loop ----
        # pair j: head 2j accumulates at psum rows 0..32, head 2j+1 at 64..96
        osb = []  # per pair (128, QH) bf16, rows 0/64 = sumexp
        for j in range(4):
            t = ppool.tile([128, QH], BF16, tag=f"osb{j}")
            osb.append(t)

        for qo, qb in Q_BLOCKS:
            for j in range(4):
                heads = (2 * j, 2 * j + 1)
                dt_i = j // 2
                o_ps = psO.tile([128, 512], F32, tag="o")
                for ti, trip in enumerate(M_TRIPS):
                    is_act = ti in ACT_TRIPS
                    e_ts = {}
                    s_tiles = {}
                    for h in heads:
                        s_ps = psS.tile([128, 2, 512], F32, tag="s")
                        s_tiles[h] = s_ps
                        if is_act:
                            for i, mc in enumerate(trip):
                                mm(
                                    s_ps[:, i, 0:qb],
                                    ident[:],
                                    bias_t[:, mc, qo : qo + qb],
                                    start=True,
                                    stop=False,
                                )
                    for i, mc in enumerate(trip):
                        for h in heads:
                            hp = (h % 4) * 32
                            mm(
                                s_tiles[h][:, i, 0:qb],
                                knT[dt_i][hp : hp + 32, mc * 128 : (mc + 1) * 128],
                                qnT[dt_i][hp : hp + 32, qo : qo + qb],
                                start=(not is_act),
                                stop=True,
                                tile_position=(hp, 0),
                            )
                    for h in heads:
                        e_t = epool.tile([128, 2, 512], BF16, tag="E")
                        if is_act:
                            act(e_t[:, :, 0:qb], s_tiles[h][:, :, 0:qb], AF.Exp)
                        else:
                            nc.vector.scalar_tensor_tensor(
                                e_t[:, :, 0:qb],
                                s_tiles[h][:, :, 0:qb],
                                1.0,
                                eb_t[:, trip[0] - N_ACT_MC : trip[0] - N_ACT_MC + 2, qo : qo + qb],
                                ALU.add,
                                ALU.mult,
                            )
                        e_ts[h] = e_t
                    for i, mc in enumerate(trip):
                        for h in heads:
                            base = 64 * (h % 2)
                            mm(
                                o_ps[base : base + 33, 0:qb],
                                vsb[mc][:, h, :],
                                e_ts[h][:, i, 0:qb],
                                start=(mc == 0),
                                stop=(mc == 7),
                                tile_position=(0, base),
                            )
                nc.scalar.copy(osb[j][:, qo : qo + qb], o_ps[:, 0:qb])

            # ---- per-qo epilogue: denominators, normalize, Wo, W_out, res ----
            sumE = wpool.tile([8, 512], BF16, tag=f"sumE{qo}")
            for h in range(8):
                r = 64 * (h % 2)
                dma(sumE[h : h + 1, 0:qb], osb[h // 2][r : r + 1, qo : qo + qb])
            lnS = wpool.tile([8, 512], F32, tag=f"lnS{qo}")
            act(lnS[:, 0:qb], sumE[:, 0:qb], AF.Ln)
            invS = wpool.tile([8, 512], F32, tag=f"invS{qo}")
            act(invS[:, 0:qb], lnS[:, 0:qb], AF.Exp, scale=-1.0)
            for j in range(4):
                ps = psO.tile([128, 512], F32, tag="o")
                mm(ps[:, 0:qb], expP[:, j, :], invS[:, 0:qb])
                nc.vector.tensor_mul(
                    osb[j][:, qo : qo + qb], osb[j][:, qo : qo + qb], ps[:, 0:qb]
                )
            o1b = []
            for dt_i in range(2):
                ps = psO.tile([128, 512], F32, tag="o")
                for j in range(4):
                    mm(
                        ps[:, 0:qb],
                        wo_aug[:, j, dt_i * 128 : (dt_i + 1) * 128],
                        osb[j][:, qo : qo + qb],
                        start=(j == 0),
                        stop=(j == 3),
                    )
                t = wpool.tile([128, 512], BF16, tag=f"o1b{dt_i}{qo}", name=f"o1b{dt_i}{qo}")
                nc.scalar.copy(t[:, 0:qb], ps[:, 0:qb])
                o1b.append(t)
            for dt_i in range(2):
                ps = psO.tile([128, 512], F32, tag="o")
                for kt in range(2):
                    mm(
                        ps[:, 0:qb],
                        wout[:, kt, dt_i * 128 : (dt_i + 1) * 128],
                        o1b[kt][:, 0:qb],
                        start=(kt == 0),
                        stop=(kt == 1),
                    )
                r1 = wpool.tile([128, 512], F32, tag=f"res{dt_i}{qo}", name=f"res{dt_i}{qo}")
                nc.vector.scalar_tensor_tensor(
                    r1[:, 0:qb],
                    ps[:, 0:qb],
                    woutb[:, dt_i, :],
                    fmapT[:, dt_i, qo : qo + qb],
                    ALU.add,
                    ALU.add,
                )
                dma(
                    out_dram[dt_i * 128 : (dt_i + 1) * 128, qo : qo + qb],
                    r1[:, 0:qb],
                )


def _prep_in_maps(inputs):
    consts = _host_constants(
        np.asarray(inputs["Wq"], np.float32),
        np.asarray(inputs["Wk"], np.float32),
        np.asarray(inputs["Wv"], np.float32),
        np.asarray(inputs["Wo"], np.float32),
        np.asarray(inputs["W_out_w"], np.float32),
        np.asarray(inputs["W_out_b"], np.float32),
        np.asarray(inputs["W_coord_w"], np.float32),
        np.asarray(inputs["W_coord_b"], np.float32),
    )
    track_tokens = np.asarray(inputs["track_tokens"], np.float32)
    feature_map = np.asarray(inputs["feature_map"], np.float32)
    feature_positions = np.asarray(inputs["feature_positions"], np.float32)
    spatial_bias = np.asarray(inputs["spatial_bias"], np.float32)

    msplit = N_ACT_MC * 128
    in_maps = []
    for c in range(N_CORES):
        t, half = c // 2, c % 2
        qsl = slice(half * QH, (half + 1) * QH)
        m = dict(consts)
        m["tokT"] = _bf(track_tokens[t].T)
        pos = feature_positions[t, qsl]
        qin = pos @ np.asarray(inputs["W_coord_w"], np.float32) + np.asarray(
            inputs["W_coord_b"], np.float32
        )
        m["ropedT"] = _bf(_rope2d_host(qin, pos).T)
        bias_cols = spatial_bias[t][:, qsl]
        m["biasT"] = _bf(bias_cols[:msplit])
        m["ebT"] = _bf(np.exp(bias_cols[msplit:]))
        m["fmapT"] = _f32(feature_map[t, qsl].T)
        in_maps.append(m)
    return in_maps


def kernel(
    track_tokens,
    feature_map,
    feature_positions,
    spatial_bias,
    Wq,
    Wk,
    Wv,
    Wo,
    W_out_w,
    W_out_b,
    W_coord_w,
    W_coord_b,
):
    global _NC_CACHE
    in_maps = _prep_in_maps(
        dict(
            track_tokens=track_tokens,
            feature_map=feature_map,
            feature_positions=feature_positions,
            spatial_bias=spatial_bias,
            Wq=Wq,
            Wk=Wk,
            Wv=Wv,
            Wo=Wo,
            W_out_w=W_out_w,
            W_out_b=W_out_b,
            W_coord_w=W_coord_w,
            W_coord_b=W_coord_b,
        )
    )

    if _NC_CACHE is None:
        _NC_CACHE = _build_nc()
    res = run_bass_kernel_spmd(_NC_CACHE, in_maps, core_ids=list(range(N_CORES)))

    out = np.empty((T, HW, DF), np.float32)
    for c in range(N_CORES):
        t, half = c // 2, c % 2
        qsl = slice(half * QH, (half + 1) * QH)
        out[t, qsl, :] = res.results[c]["out"].T
    return out
